# revision 14
# baseline (speedup 1.0000x reference)
"""Trainium2 Bass kernel for nn_MGA_50766513439346 (gnn_message_passing).

Reference math (per node n, E=64, T=3 behavior types):
  stage(key, Q, W, b): score_t = key.Wk + q_t.Wq + b ; a = softmax_t(score) ;
                       out = sum_t a_t * q_t
  out = stage(stage(buy, [view,cart,buy], W0, b0), [view_buy,cart_buy,buy_buy], W1, b1)

Key identity: the key.Wk term and bias b are constant along the softmax axis t,
so they cancel exactly in softmax.  Hence stage-1's output (the stage-2 "key")
never affects the final output, which reduces to a single attention over the
three *_buy tables with weights softmax_t(q_t . Wq1):

  s_t   = q_t . W1[:, 64:128]          (t in {view_buy, cart_buy, buy_buy})
  e_t   = exp(s_t)                      (|s| < ~6, no overflow; max-sub skipped)
  out   = (sum_t e_t * q_t) / (sum_t e_t)

Sharding: rows N=500000 split evenly across 8 cores (62500 each, zero-padded to
62592 = 489*128); weights replicated.  No cross-device communication.

Layout per core: rows on SBUF partitions (128/tile), G row-groups x T tables
packed in the free axis -> tiles [128, G, 3, 64].  DVE does mult/reduce passes,
ACT does exp.  All fp32.
"""

from contextlib import ExitStack

import numpy as np

import concourse.bass as bass
import bass_rust as _bass_rust
import concourse.tile as tile
from concourse import mybir
from concourse.bass_utils import run_bass_kernel_spmd

EMB = 64
T = 3
N_TOTAL = 500000
N_CORES = 8
N_PER_CORE = N_TOTAL // N_CORES          # 62500
P = 128
N_GROUPS = (N_PER_CORE + P - 1) // P     # 489
R_PAD = N_GROUPS * P                     # 62592
G_MAIN = 24                              # row-groups per big tile

F32 = mybir.dt.float32


def _tile_plan(r_pad=R_PAD, g_main=G_MAIN):
    """(row_offset, n_groups) covering r_pad/P groups."""
    n_groups = r_pad // P
    plan = []
    g_done = 0
    while g_done < n_groups:
        g = min(g_main, n_groups - g_done)
        plan.append((g_done * P, g))
        g_done += g
    return plan


def _build_program(r_pad=R_PAD, g_main=G_MAIN, loop_reps=1):
    nc = bass.Bass()
    vb = nc.declare_dram_parameter("vb", [r_pad, EMB], F32, isOutput=False)
    cb = nc.declare_dram_parameter("cb", [r_pad, EMB], F32, isOutput=False)
    bb = nc.declare_dram_parameter("bb", [r_pad, EMB], F32, isOutput=False)
    w1 = nc.declare_dram_parameter("w1", [1, 2 * EMB], F32, isOutput=False)
    out = nc.declare_dram_parameter("out", [r_pad, EMB], F32, isOutput=True)

    tables = (vb, cb, bb)

    with tile.TileContext(nc) as tc, ExitStack() as ctx:
        singles = ctx.enter_context(tc.tile_pool(name="singles", bufs=1))
        qpool = ctx.enter_context(tc.tile_pool(name="q", bufs=3))
        tpool = ctx.enter_context(tc.tile_pool(name="tmp", bufs=2))
        opool = ctx.enter_context(tc.tile_pool(name="o", bufs=2))
        spool = ctx.enter_context(tc.tile_pool(name="s", bufs=4))

        # Wq1 = W1[0, 64:128], replicated to [128, T*EMB] (same 64 weights
        # for each of the T tables, all partitions).
        TE = T * EMB
        wq_rep = singles.tile([P, TE], F32)
        wq_src = w1[0:1, EMB : 2 * EMB]
        wq_bcast = bass.AP(
            tensor=wq_src.tensor,
            offset=wq_src.offset,
            ap=[[0, P], [0, T], [1, EMB]],
        )
        nc.gpsimd.dma_start(out=wq_rep[:, :].rearrange("p (t e) -> p t e", e=EMB),
                            in_=wq_bcast)

        wq64 = wq_rep[:, 0:EMB]

        def body():
            for row0, g in _tile_plan(r_pad, g_main):
                rows = g * P
                # Per-table tiles/ops: each instruction waits on at most one
                # DMA producer (walrus caps sync-wait commands per inst).
                qs = [qpool.tile([P, g, EMB], F32, tag=f"q{t}", name=f"q{t}") for t in range(T)]
                for t, tbl in enumerate(tables):
                    src = tbl[row0 : row0 + rows, :].rearrange("(g p) e -> p g e", p=P)
                    nc.sync.dma_start(out=qs[t], in_=src)

                # s3[p, g, t] = sum_e q_t*wq
                tmp = tpool.tile([P, g, T, EMB], F32, tag="tmp")
                for t in range(T):
                    nc.vector.tensor_mul(
                        tmp[:, :, t, :], qs[t],
                        wq64[:, None, :].broadcast_to([P, g, EMB]),
                    )
                s3 = spool.tile([P, g * T], F32, tag="s3")
                nc.vector.reduce_sum(
                    out=s3,
                    in_=tmp.rearrange("p g t e -> p (g t) e"),
                    axis=mybir.AxisListType.X,
                )

                # e3 = exp(s3); denom = sum_t e3 ; r = 1/denom ; a3 = e3*r
                e3 = spool.tile([P, g * T], F32, tag="e3")
                nc.scalar.activation(
                    out=e3, in_=s3, func=mybir.ActivationFunctionType.Exp
                )
                denom = spool.tile([P, g], F32, tag="denom")
                nc.vector.reduce_sum(
                    out=denom,
                    in_=e3.rearrange("p (g t) -> p g t", t=T),
                    axis=mybir.AxisListType.X,
                )
                r = spool.tile([P, g], F32, tag="r")
                nc.vector.reciprocal(out=r, in_=denom)
                a3 = spool.tile([P, g * T], F32, tag="a3")
                nc.vector.tensor_mul(
                    a3.rearrange("p (g t) -> p g t", t=T),
                    e3.rearrange("p (g t) -> p g t", t=T),
                    r[:, :, None].broadcast_to([P, g, T]),
                )
                a3v = a3.rearrange("p (g t) -> p g t", t=T)

                # o = sum_t a3[p,g,t] * q_t[p,g,e]
                wt = tpool.tile([P, g, T, EMB], F32, tag="wt")
                for t in range(T):
                    nc.vector.tensor_mul(
                        wt[:, :, t, :], qs[t],
                        a3v[:, :, t : t + 1].broadcast_to([P, g, EMB]),
                    )
                o = opool.tile([P, g, EMB], F32, tag="o")
                nc.vector.tensor_add(o, wt[:, :, 0, :], wt[:, :, 1, :])
                nc.vector.tensor_add(o, o, wt[:, :, 2, :])

                dst = out[row0 : row0 + rows, :].rearrange("(g p) e -> p g e", p=P)
                nc.sync.dma_start(out=dst, in_=o)

        if loop_reps > 1:
            with tc.For_i(0, loop_reps, 1):
                body()
        else:
            body()

    # Walrus codegen allows at most one sync-wait per instruction; this pass
    # splits multi-waits into EventSemaphore instructions (normally run by
    # Bacc.compile, which we don't use).
    _bass_rust.generate_event_semaphores(nc)
    return nc


def _shard_pad(arr, core):
    sl = arr[core * N_PER_CORE : (core + 1) * N_PER_CORE]
    if R_PAD == N_PER_CORE:
        return np.ascontiguousarray(sl, dtype=np.float32)
    out = np.zeros((R_PAD, EMB), dtype=np.float32)
    out[:N_PER_CORE] = sl
    return out


def run(inputs, loop_reps=1):
    """Returns full_output [N,64] fp32."""
    view_buy = np.asarray(inputs["view_buy"], dtype=np.float32)
    cart_buy = np.asarray(inputs["cart_buy"], dtype=np.float32)
    buy_buy = np.asarray(inputs["buy_buy"], dtype=np.float32)
    w1 = np.ascontiguousarray(np.asarray(inputs["W1"], dtype=np.float32))

    nc = _build_program(loop_reps=loop_reps)
    in_maps = [
        {
            "vb": _shard_pad(view_buy, c),
            "cb": _shard_pad(cart_buy, c),
            "bb": _shard_pad(buy_buy, c),
            "w1": w1,
        }
        for c in range(N_CORES)
    ]
    res = run_bass_kernel_spmd(nc, in_maps, list(range(N_CORES)))
    out = np.concatenate(
        [res.results[c]["out"][:N_PER_CORE] for c in range(N_CORES)], axis=0
    )
    return out


def kernel(**inputs) -> np.ndarray:
    return run(inputs)


if __name__ == "__main__":
    rng = np.random.default_rng(0)
    n = N_TOTAL
    demo = {
        name: rng.standard_normal((n, EMB), dtype=np.float32)
        for name in ("view_buy", "cart_buy", "buy_buy")
    }
    demo["W1"] = (rng.standard_normal((1, 2 * EMB)) * 0.1).astype(np.float32)
    out, t = run(demo)
    print(out.shape, out.dtype, t)
